# revision 30
# baseline (speedup 1.0000x reference)
"""Contrastive loss (GRACE-style) on 8 Trainium2 NeuronCores — symmetric fp8.

loss = sum_i 0.5*(l1_i + l2_i)
  l1 = log(rowsum(exp(h1@h1.T/t)) + rowsum(exp(h1@h2.T/t)) - diag_refl) - log(diag_bet)
  l2 = same with h1<->h2;  h = z / ||z||_row,  t = 0.2

Device computes, in fp8 DoubleRow with a single array rh = fp8(16*h) serving
both matmul operands (exp scale c = 1/(256*t) is a constant immediate):
  - refl1/refl2 exploiting symmetry: row block a computes the contiguous
    (mod N) column window [128a, 128a + W), W = 33 blocks for a<32 else 32.
    Row sums come free via ACT accum_out; the transposed halves are credited
    by elementwise-accumulating exp tiles into colacc (DVE) and partition-
    reducing at the end (GPSIMD axis=C). Every off-diag window block is
    credited; the diag block is included in the window but not credited.
  - between1: row-sharded, each unit computes all N columns. Row sums via
    accum_out; column sums (= between2 row sums, exact transpose since the
    same fp8 operand pair feeds both) via bcolacc + partition-reduce.
Work is sharded round-robin: core r owns row blocks a ≡ r (mod 8). Each core
receives its input pre-rotated by 128r columns (and extended by 4224 wrap
columns), so all cores execute the identical SPMD program; the host un-rotates
the column-indexed outputs. Host (numpy, O(N*D)) applies exact diagonal
corrections + logs in float64.
"""

import numpy as np
import ml_dtypes

N = 8192
D = 512
NCORES = 8
P = 128
KD = D // P          # 4 k-subtiles; DoubleRow consumes them in pairs
NB = N // P          # 64 row blocks
UPC = NB // NCORES   # 8 units per core per matrix
TAU = 0.2
RS = 16.0            # fp8 h pre-scale; keeps values in the normal range
CEXP = 1.0 / (RS * RS * TAU)
WEXT = 33 * P        # wrap extension: widest window
WN = N + WEXT        # per-core input width (rotated + extended)

# unit u handles row block a = r + 8u; a<32 <=> u<4 (window incl dist-32 tail)
UNIT_CHUNKS = [(1536, 1536, 1152)] * 4 + [(2048, 2048)] * 4
BCH = 2048           # between-pass chunk width

_CACHE = {}


def _fold_spans(start, end):
    """Split local span [start, end) at the N wrap.

    Returns (fold_start, fold_end, off) pieces where off is the piece's
    offset from the span start in the unfolded (source buffer) coordinates.
    """
    out = []
    if start < N:
        out.append((start, min(end, N), 0))
    if end > N:
        s = max(start, N)
        out.append((s - N, end - N, s - start))
    return out


def _build(repeat=1, loop=None, no_creduce=False, no_accum=False):
    import concourse.tile as tile
    from concourse import bacc, mybir

    f32 = mybir.dt.float32
    bf16 = mybir.dt.bfloat16
    fp8 = mybir.dt.float8e4
    AF = mybir.ActivationFunctionType
    ALU = mybir.AluOpType
    AX = mybir.AxisListType
    DR = mybir.MatmulPerfMode.DoubleRow

    nc = bacc.Bacc("TRN2", target_bir_lowering=False, debug=False,
                   num_devices=NCORES)

    rhw1 = nc.dram_tensor("rhw1", [D, WN], fp8, kind="ExternalInput")
    rhw2 = nc.dram_tensor("rhw2", [D, WN], fp8, kind="ExternalInput")
    racc1 = nc.dram_tensor("racc1", [P, 3 * UPC], f32, kind="ExternalOutput")
    racc2 = nc.dram_tensor("racc2", [P, 3 * UPC], f32, kind="ExternalOutput")
    bacc_d = nc.dram_tensor("bacc", [P, 4 * UPC], f32, kind="ExternalOutput")
    ccol1 = nc.dram_tensor("ccol1", [N], bf16, kind="ExternalOutput")
    ccol2 = nc.dram_tensor("ccol2", [N], bf16, kind="ExternalOutput")
    bcol = nc.dram_tensor("bcol", [N], bf16, kind="ExternalOutput")

    rhw1v = rhw1.rearrange("(k p) n -> p k n", p=P)
    rhw2v = rhw2.rearrange("(k p) n -> p k n", p=P)

    with tile.TileContext(nc) as tc:
        with (
            tc.tile_pool(name="singles", bufs=1) as singles,
            tc.tile_pool(name="es", bufs=4) as esp,
            tc.tile_pool(name="st", bufs=2) as stp,
            tc.tile_pool(name="ps", bufs=2, space="PSUM") as psp,
        ):
            rh1s = singles.tile([P, KD, WN], fp8, tag="rh1s")
            rh2s = singles.tile([P, KD, WN], fp8, tag="rh2s")
            colacc1 = singles.tile([P, N], bf16, tag="colacc1")
            colacc2 = singles.tile([P, N], bf16, tag="colacc2")
            bcolacc = singles.tile([P, N], bf16, tag="bcolacc")
            racc1s = singles.tile([P, 3 * UPC], f32, tag="racc1s")
            racc2s = singles.tile([P, 3 * UPC], f32, tag="racc2s")
            baccs = singles.tile([P, 4 * UPC], f32, tag="baccs")
            ones = singles.tile([P, 1], bf16, tag="ones")
            nc.vector.memset(racc1s, 0.0)
            nc.vector.memset(racc2s, 0.0)
            nc.vector.memset(ones, 1.0)

            # input DMAs, sliced so compute starts after the first pieces land
            NPC = 7  # 6x2048 + 128
            for t, (sb, dv) in enumerate(((rh1s, rhw1v), (rh2s, rhw2v))):
                for i in range(NPC):
                    a, b = i * 2048, min((i + 1) * 2048, WN)
                    nc.sync.dma_start(out=sb[:, :, a:b], in_=dv[:, :, a:b])

            def _mm_chunk(rhs_src, lhs_src, u, ps, c0, csize, base):
                lhsT0 = 1024 * u
                for kp in range(2):
                    lhsT = lhs_src[:, 2 * kp:2 * kp + 2, lhsT0:lhsT0 + P]
                    for off in range(0, csize, 512):
                        w = min(512, csize - off)
                        s = base + c0 + off
                        nc.tensor.matmul(
                            ps[:, off:off + w], lhsT=lhsT,
                            rhs=rhs_src[:, 2 * kp:2 * kp + 2, s:s + w],
                            start=(kp == 0), stop=(kp == 1), perf_mode=DR)

            # greedy 3-engine load balancer (cumulative busy-time in us):
            # each offloadable op goes to the engine that stays least loaded.
            # ns/col rates: DVE add 0.52 (2x bf16) / copy 0.26 (4x); GPSIMD
            # add 1.98 (0.42 eff) / copy 1.39 (0.6 eff); ACT 0.833/elem+352c.
            bal = {"act": 0.0, "dve": 0.0, "gp": 0.0}

            def _accum(colacc_dst, src_es, w, state):
                """colacc_dst (+)= src_es on DVE or GPSIMD, whichever is
                less loaded after taking the op."""
                cd = w * (0.52 if state else 0.26) / 1000
                cg = w * (1.98 if state else 1.39) / 1000
                if bal["dve"] + cd <= bal["gp"] + cg:
                    eng = nc.vector
                    bal["dve"] += cd
                else:
                    eng = nc.gpsimd
                    bal["gp"] += cg
                if state:
                    eng.tensor_add(colacc_dst, colacc_dst, src_es)
                else:
                    eng.tensor_copy(colacc_dst, src_es)

            def _credit(colacc, es, spans, touched):
                for fs, fe, eo in spans:
                    b0, b1 = fs // P, fe // P
                    b = b0
                    while b < b1:
                        state = touched[b]
                        e = b
                        while e < b1 and touched[e] == state:
                            touched[e] = True
                            e += 1
                        _accum(colacc[:, b * P:e * P],
                               es[:, eo + (b - b0) * P: eo + (e - b0) * P],
                               (e - b) * P, state)
                        b = e

            def _exp_rowsum(ps, es, csize, acc_slot):
                """exp from PSUM into es; rowsum either fused on ACT
                (accum_out, ~+0.61ns/elem measured) or as DVE fold+reduce
                (0.78ns/elem)."""
                ca = csize * 0.00061
                cd = csize * 0.00078
                bal["act"] += (csize + 352) * 0.000833
                if bal["act"] + ca <= bal["dve"] + cd:
                    bal["act"] += ca
                    nc.scalar.activation(out=es[:, :csize], in_=ps[:, :csize],
                                         func=AF.Exp, scale=CEXP,
                                         accum_out=acc_slot)
                else:
                    bal["dve"] += cd
                    nc.scalar.activation(out=es[:, :csize], in_=ps[:, :csize],
                                         func=AF.Exp, scale=CEXP)
                    h = csize // 2
                    fold = stp.tile([P, 1024], bf16, tag="fold", name="fold")
                    nc.vector.tensor_add(fold[:, :h], es[:, :h],
                                         es[:, h:csize])
                    nc.vector.tensor_reduce(acc_slot, fold[:, :h],
                                            axis=AX.X, op=ALU.add)

            def _creduce(src, dst_dram, phase_tag):
                # partition-sum via ones-matmul; 16 x 512-col segments packed
                # into 4 partition strips (0/32/64/96) of one PSUM tile, then
                # one strided DVE copy extracts [4, 2048] -> DMA out.
                if no_creduce:
                    return
                cps = psp.tile([P, 2048], f32, tag="ps",
                               name=f"cps_{phase_tag}")
                for s in range(16):
                    q = 32 * (s // 4)
                    off = 512 * (s % 4)
                    nc.tensor.matmul(
                        cps[q:q + 1, off:off + 512], lhsT=ones,
                        rhs=src[:, 512 * s:512 * (s + 1)],
                        start=True, stop=True, skip_group_check=True,
                        tile_position=(0, q))
                st = stp.tile([1, N], bf16, tag="st", bufs=1,
                              name=f"st_{phase_tag}")
                for g in range(4):
                    dst = st[0:1, 2048 * g:2048 * (g + 1)]
                    srcp = cps[32 * g:32 * g + 1, :]
                    if bal["dve"] + 2.13 <= bal["act"] + 2.0:
                        bal["dve"] += 2.13
                        nc.vector.tensor_copy(dst, srcp)
                    else:
                        bal["act"] += 2.0
                        nc.scalar.copy(dst, srcp)
                nc.sync.dma_start(
                    out=dst_dram.rearrange("(o c) -> o c", o=1), in_=st)

            def _refl_units(rhs_s, raccs, colacc, touched, u_from, u_to):
                for u in range(u_from, u_to):
                    chunks = UNIT_CHUNKS[u]
                    c0 = 0
                    for ci, csize in enumerate(chunks):
                        ps = psp.tile([P, 2048], f32, tag="ps", name="psr")
                        _mm_chunk(rhs_s, rhs_s, u, ps, c0, csize, 1024 * u)
                        es = esp.tile([P, 2048], bf16, tag="es", name="esr")
                        _exp_rowsum(ps, es, csize,
                                    raccs[:, 3 * u + ci:3 * u + ci + 1])
                        lo = max(c0, P)  # exclude diag block from credits
                        hi = c0 + csize
                        if lo < hi:
                            spans = [
                                (fs, fe, lo - c0 + eo)
                                for fs, fe, eo in _fold_spans(
                                    1024 * u + lo, 1024 * u + hi)]
                            _credit(colacc, es, spans, touched)
                        c0 += csize

            def _between_units(u_from, u_to):
                for u in range(u_from, u_to):
                    for ci in range(N // BCH):
                        c0 = ci * BCH
                        ps = psp.tile([P, 2048], f32, tag="ps", name="psb")
                        _mm_chunk(rh2s, rh1s, u, ps, 0, BCH, c0)
                        es = esp.tile([P, 2048], bf16, tag="es", name="esb")
                        _exp_rowsum(ps, es, BCH,
                                    baccs[:, 4 * u + ci:4 * u + ci + 1])
                        _accum(bcolacc[:, c0:c0 + BCH], es, BCH, u > 0)

            def _main_body():
                bal["act"] = bal["dve"] = bal["gp"] = 0.0
                t1 = [False] * NB
                t2 = [False] * NB
                _refl_units(rh1s, racc1s, colacc1, t1, 0, UPC)
                # creduce(colacc1) after 3 between units: by then the refl1
                # credit tail (GPSIMD) has drained, so no cross-phase stall
                _between_units(0, 3)
                _creduce(colacc1, ccol1, "ccol1")
                _between_units(3, UPC)
                _refl_units(rh2s, racc2s, colacc2, t2, 0, 2)
                _creduce(bcolacc, bcol, "bcol")
                _refl_units(rh2s, racc2s, colacc2, t2, 2, UPC)
                _creduce(colacc2, ccol2, "ccol2")
                if not no_accum:
                    nc.sync.dma_start(out=racc1[:, :], in_=racc1s)
                    nc.sync.dma_start(out=racc2[:, :], in_=racc2s)
                    nc.sync.dma_start(out=bacc_d[:, :], in_=baccs)

            if loop is not None:
                with tc.For_i(0, loop):
                    _main_body()
            else:
                for _rep in range(repeat):
                    _main_body()

    nc.compile()
    return nc


def _get_nc(repeat=1, loop=None, **kw):
    key = ("nc", repeat, loop, tuple(sorted(kw.items())))
    if key not in _CACHE:
        _CACHE[key] = _build(repeat, loop=loop, **kw)
    return _CACHE[key]


def _host_prep(z1, z2):
    fp8 = ml_dtypes.float8_e4m3
    z1 = np.asarray(z1, dtype=np.float32)
    z2 = np.asarray(z2, dtype=np.float32)
    n1 = np.maximum(np.linalg.norm(z1, axis=1), 1e-12)
    n2 = np.maximum(np.linalg.norm(z2, axis=1), 1e-12)
    h1 = z1 / n1[:, None]
    h2 = z2 / n2[:, None]
    r1_8 = (h1 * RS).astype(fp8)
    r2_8 = (h2 * RS).astype(fp8)
    return r1_8, r2_8, h1, h2


def make_in_maps(z1, z2):
    r1_8, r2_8, _, _ = _host_prep(z1, z2)
    d1 = np.concatenate([r1_8.T, r1_8.T], axis=1)
    d2 = np.concatenate([r2_8.T, r2_8.T], axis=1)
    in_maps = []
    for r in range(NCORES):
        o = P * r
        in_maps.append({
            "rhw1": np.ascontiguousarray(d1[:, o:o + WN]),
            "rhw2": np.ascontiguousarray(d2[:, o:o + WN]),
        })
    return in_maps


def kernel(z1, z2):
    from concourse.bass_utils import run_bass_kernel_spmd

    r1_8, r2_8, h1, h2 = _host_prep(z1, z2)
    in_maps = make_in_maps(z1, z2)

    nc = _get_nc()
    res = run_bass_kernel_spmd(nc, in_maps, core_ids=list(range(NCORES)))

    S1 = np.zeros((NB, P), dtype=np.float64)
    S2 = np.zeros((NB, P), dtype=np.float64)
    Sb1 = np.zeros((NB, P), dtype=np.float64)
    C1 = np.zeros(N, dtype=np.float64)
    C2 = np.zeros(N, dtype=np.float64)
    Sb2 = np.zeros(N, dtype=np.float64)
    for r in range(NCORES):
        out = res.results[r]
        ra1 = out["racc1"].astype(np.float64).reshape(P, UPC, 3).sum(-1)
        ra2 = out["racc2"].astype(np.float64).reshape(P, UPC, 3).sum(-1)
        ba = out["bacc"].astype(np.float64).reshape(P, UPC, 4).sum(-1)
        for u in range(UPC):
            a = r + NCORES * u
            S1[a] += ra1[:, u]
            S2[a] += ra2[:, u]
            Sb1[a] += ba[:, u]
        C1 += np.roll(out["ccol1"].astype(np.float64), P * r)
        C2 += np.roll(out["ccol2"].astype(np.float64), P * r)
        Sb2 += np.roll(out["bcol"].astype(np.float64), P * r)

    S1 = S1.reshape(-1) + C1
    S2 = S2.reshape(-1) + C2
    Sb1 = Sb1.reshape(-1)

    # exact diagonal corrections from the same fp8 data the device used
    r1f = r1_8.astype(np.float64)
    r2f = r2_8.astype(np.float64)
    q1 = CEXP * (r1f * r1f).sum(1)
    q2 = CEXP * (r2f * r2f).sum(1)
    v5 = (h1.astype(np.float64) * h2.astype(np.float64)).sum(1) / TAU

    d1 = S1 + Sb1 - np.exp(q1)
    d2 = S2 + Sb2 - np.exp(q2)
    loss = 0.5 * (np.log(d1) + np.log(d2)) - v5
    return np.float32(loss.sum())


# revision 40
# speedup vs baseline: 1.8862x; 1.8862x over previous
"""Contrastive loss (GRACE-style) on 8 Trainium2 NeuronCores — symmetric fp8.

loss = sum_i 0.5*(l1_i + l2_i)
  l1 = log(rowsum(exp(h1@h1.T/t)) + rowsum(exp(h1@h2.T/t)) - diag_refl) - log(diag_bet)
  l2 = same with h1<->h2;  h = z / ||z||_row,  t = 0.2

Device computes, in fp8 DoubleRow with a single array rh = fp8(16*h) serving
both matmul operands (exp scale c = 1/(256*t) is a constant immediate):
  - refl1/refl2 exploiting symmetry: row block a computes the contiguous
    (mod N) column window [128a, 128a + W), W = 33 blocks for a<32 else 32.
    Row sums come free via ACT accum_out; the transposed halves are credited
    by elementwise-accumulating exp tiles into colacc (DVE) and partition-
    reducing at the end (GPSIMD axis=C). Every off-diag window block is
    credited; the diag block is included in the window but not credited.
  - between1: row-sharded, each unit computes all N columns. Row sums via
    accum_out; column sums (= between2 row sums, exact transpose since the
    same fp8 operand pair feeds both) via bcolacc + partition-reduce.
Work is sharded round-robin: core r owns row blocks a ≡ r (mod 8). Each core
receives its input pre-rotated by 128r columns (and extended by 4224 wrap
columns), so all cores execute the identical SPMD program; the host un-rotates
the column-indexed outputs. Host (numpy, O(N*D)) applies exact diagonal
corrections + logs in float64.
"""

import numpy as np
import ml_dtypes

N = 8192
D = 512
NCORES = 8
P = 128
KD = D // P          # 4 k-subtiles; DoubleRow consumes them in pairs
NB = N // P          # 64 row blocks
UPC = NB // NCORES   # 8 units per core per matrix
TAU = 0.2
RS = 16.0            # fp8 h pre-scale; keeps values in the normal range
CEXP = 1.0 / (RS * RS * TAU)
WEXT = 33 * P        # wrap extension: widest window
WN = N + WEXT        # per-core input width (rotated + extended)

# unit u handles row block a = r + 8u; a<32 <=> u<4 (window incl dist-32 tail)
UNIT_CHUNKS = [(1536, 1536, 1152)] * 4 + [(2048, 2048)] * 4
BCH = 2048           # between-pass chunk width

_CACHE = {}


def _fold_spans(start, end):
    """Split local span [start, end) at the N wrap.

    Returns (fold_start, fold_end, off) pieces where off is the piece's
    offset from the span start in the unfolded (source buffer) coordinates.
    """
    out = []
    if start < N:
        out.append((start, min(end, N), 0))
    if end > N:
        s = max(start, N)
        out.append((s - N, end - N, s - start))
    return out


def _build(repeat=1, loop=None, no_creduce=False, no_accum=False, force_dve=True, mm_only=False, rowsum='accum', esbufs=6, extr='bal'):
    import concourse.tile as tile
    from concourse import bacc, mybir

    f32 = mybir.dt.float32
    bf16 = mybir.dt.bfloat16
    fp8 = mybir.dt.float8e4
    AF = mybir.ActivationFunctionType
    ALU = mybir.AluOpType
    AX = mybir.AxisListType
    DR = mybir.MatmulPerfMode.DoubleRow

    nc = bacc.Bacc("TRN2", target_bir_lowering=False, debug=False,
                   num_devices=NCORES)

    rhw1 = nc.dram_tensor("rhw1", [D, WN], fp8, kind="ExternalInput")
    rhw2 = nc.dram_tensor("rhw2", [D, WN], fp8, kind="ExternalInput")
    racc1 = nc.dram_tensor("racc1", [P, 3 * UPC], f32, kind="ExternalOutput")
    racc2 = nc.dram_tensor("racc2", [P, 3 * UPC], f32, kind="ExternalOutput")
    bacc_d = nc.dram_tensor("bacc", [P, 4 * UPC], f32, kind="ExternalOutput")
    ccol1 = nc.dram_tensor("ccol1", [N], bf16, kind="ExternalOutput")
    ccol2 = nc.dram_tensor("ccol2", [N], bf16, kind="ExternalOutput")
    bcol = nc.dram_tensor("bcol", [N], bf16, kind="ExternalOutput")

    rhw1v = rhw1.rearrange("(k p) n -> p k n", p=P)
    rhw2v = rhw2.rearrange("(k p) n -> p k n", p=P)

    with tile.TileContext(nc) as tc:
        with (
            tc.tile_pool(name="singles", bufs=1) as singles,
            tc.tile_pool(name="es", bufs=esbufs) as esp,
            tc.tile_pool(name="st", bufs=2) as stp,
            tc.tile_pool(name="ps", bufs=2, space="PSUM") as psp,
        ):
            rh1s = singles.tile([P, KD, WN], fp8, tag="rh1s")
            rh2s = singles.tile([P, KD, WN], fp8, tag="rh2s")
            colacc1 = singles.tile([P, N], bf16, tag="colacc1")
            colacc2 = singles.tile([P, N], bf16, tag="colacc2")
            bcolacc = singles.tile([P, N], bf16, tag="bcolacc")
            racc1s = singles.tile([P, 3 * UPC], f32, tag="racc1s")
            racc2s = singles.tile([P, 3 * UPC], f32, tag="racc2s")
            baccs = singles.tile([P, 4 * UPC], f32, tag="baccs")
            ones = singles.tile([P, 1], bf16, tag="ones")
            nc.vector.memset(racc1s, 0.0)
            nc.vector.memset(racc2s, 0.0)
            nc.vector.memset(ones, 1.0)

            # input DMAs, sliced so compute starts after the first pieces
            # land; the two tensors ride separate HWDGE queues in parallel
            cuts = [0, 512, 1024, 2048, 3072, 4224, 6144, 8192, 10240, WN]
            for sb, dv in ((rh1s, rhw1v), (rh2s, rhw2v)):
                for a, b in zip(cuts[:-1], cuts[1:]):
                    nc.sync.dma_start(out=sb[:, :, a:b], in_=dv[:, :, a:b])

            def _mm_chunk(rhs_src, lhs_src, u, ps, c0, csize, base):
                lhsT0 = 1024 * u
                for kp in range(2):
                    lhsT = lhs_src[:, 2 * kp:2 * kp + 2, lhsT0:lhsT0 + P]
                    for off in range(0, csize, 512):
                        w = min(512, csize - off)
                        s = base + c0 + off
                        nc.tensor.matmul(
                            ps[:, off:off + w], lhsT=lhsT,
                            rhs=rhs_src[:, 2 * kp:2 * kp + 2, s:s + w],
                            start=(kp == 0), stop=(kp == 1), perf_mode=DR)

            # greedy 3-engine load balancer (cumulative busy-time in us):
            # each offloadable op goes to the engine that stays least loaded.
            # ns/col rates: DVE add 0.52 (2x bf16) / copy 0.26 (4x); GPSIMD
            # add 1.98 (0.42 eff) / copy 1.39 (0.6 eff); ACT 0.833/elem+352c.
            bal = {"act": 0.0, "dve": 0.0, "gp": 0.0}

            def _accum(colacc_dst, src_es, w, state):
                """colacc_dst (+)= src_es on DVE or GPSIMD, whichever is
                less loaded after taking the op."""
                cd = w * (0.567 if state else 0.28) / 1000
                cg = w * (1.98 if state else 1.39) / 1000
                if force_dve or bal["dve"] + cd <= bal["gp"] + cg:
                    eng = nc.vector
                    bal["dve"] += cd
                else:
                    eng = nc.gpsimd
                    bal["gp"] += cg
                if state:
                    eng.tensor_add(colacc_dst, colacc_dst, src_es)
                else:
                    eng.tensor_copy(colacc_dst, src_es)

            def _credit(colacc, es, spans, touched):
                for fs, fe, eo in spans:
                    b0, b1 = fs // P, fe // P
                    b = b0
                    while b < b1:
                        state = touched[b]
                        e = b
                        while e < b1 and touched[e] == state:
                            touched[e] = True
                            e += 1
                        _accum(colacc[:, b * P:e * P],
                               es[:, eo + (b - b0) * P: eo + (e - b0) * P],
                               (e - b) * P, state)
                        b = e

            def _exp_rowsum(ps, es, csize, acc_slot):
                """exp from PSUM into es; rowsum either fused on ACT
                (accum_out, +0.26ns/elem measured) or as a plain DVE 1x
                reduce (1.07ns/elem measured); greedy pick by load."""
                ca = csize * 0.00026
                cd = csize * 0.00107 + 0.06
                bal["act"] += (csize + 352) * 0.000833
                if (rowsum == "accum"
                        or (rowsum == "bal"
                            and bal["act"] + ca <= bal["dve"] + cd)):
                    bal["act"] += ca
                    nc.scalar.activation(out=es[:, :csize], in_=ps[:, :csize],
                                         func=AF.Exp, scale=CEXP,
                                         accum_out=acc_slot)
                else:
                    bal["dve"] += cd
                    nc.scalar.activation(out=es[:, :csize], in_=ps[:, :csize],
                                         func=AF.Exp, scale=CEXP)
                    nc.vector.tensor_reduce(acc_slot, es[:, :csize],
                                            axis=AX.X, op=ALU.add)

            def _creduce(src, dst_dram, phase_tag):
                # partition-sum via ones-matmul; 16 x 512-col segments packed
                # into 4 partition strips (0/32/64/96) of one PSUM tile, then
                # one strided DVE copy extracts [4, 2048] -> DMA out.
                if no_creduce or mm_only:
                    return
                cps = psp.tile([P, 2048], f32, tag="ps",
                               name=f"cps_{phase_tag}")
                for s in range(16):
                    q = 32 * (s // 4)
                    off = 512 * (s % 4)
                    nc.tensor.matmul(
                        cps[q:q + 1, off:off + 512], lhsT=ones,
                        rhs=src[:, 512 * s:512 * (s + 1)],
                        start=True, stop=True, skip_group_check=True,
                        tile_position=(0, q))
                st = stp.tile([1, N], bf16, tag="st", bufs=1,
                              name=f"st_{phase_tag}")
                for g in range(4):
                    dst = st[0:1, 2048 * g:2048 * (g + 1)]
                    srcp = cps[32 * g:32 * g + 1, :]
                    if extr == "dve" or (extr == "bal" and
                            bal["dve"] + 2.13 <= bal["act"] + 2.0):
                        bal["dve"] += 2.13
                        nc.vector.tensor_copy(dst, srcp)
                    else:
                        bal["act"] += 2.0
                        nc.scalar.copy(dst, srcp)
                nc.sync.dma_start(
                    out=dst_dram.rearrange("(o c) -> o c", o=1), in_=st)

            def _refl_units(rhs_s, raccs, colacc, touched, u_from, u_to):
                for u in range(u_from, u_to):
                    chunks = UNIT_CHUNKS[u]
                    c0 = 0
                    for ci, csize in enumerate(chunks):
                        ps = psp.tile([P, 2048], f32, tag="ps", name="psr")
                        _mm_chunk(rhs_s, rhs_s, u, ps, c0, csize, 1024 * u)
                        if mm_only:
                            c0 += csize
                            continue
                        es = esp.tile([P, 2048], bf16, tag="es", name="esr")
                        _exp_rowsum(ps, es, csize,
                                    raccs[:, 3 * u + ci:3 * u + ci + 1])
                        lo = max(c0, P)  # exclude diag block from credits
                        hi = c0 + csize
                        if lo < hi:
                            spans = [
                                (fs, fe, lo - c0 + eo)
                                for fs, fe, eo in _fold_spans(
                                    1024 * u + lo, 1024 * u + hi)]
                            _credit(colacc, es, spans, touched)
                        c0 += csize

            def _between_units(u_from, u_to):
                for u in range(u_from, u_to):
                    for ci in range(N // BCH):
                        c0 = ci * BCH
                        ps = psp.tile([P, 2048], f32, tag="ps", name="psb")
                        _mm_chunk(rh2s, rh1s, u, ps, 0, BCH, c0)
                        if mm_only:
                            continue
                        es = esp.tile([P, 2048], bf16, tag="es", name="esb")
                        _exp_rowsum(ps, es, BCH,
                                    baccs[:, 4 * u + ci:4 * u + ci + 1])
                        _accum(bcolacc[:, c0:c0 + BCH], es, BCH, u > 0)

            def _main_body():
                bal["act"] = bal["dve"] = bal["gp"] = 0.0
                t1 = [False] * NB
                t2 = [False] * NB
                _refl_units(rh1s, racc1s, colacc1, t1, 0, UPC)
                # creduce(colacc1) after 3 between units: by then the refl1
                # credit tail (GPSIMD) has drained, so no cross-phase stall
                _between_units(0, 3)
                _creduce(colacc1, ccol1, "ccol1")
                _between_units(3, UPC)
                _refl_units(rh2s, racc2s, colacc2, t2, 0, 2)
                _creduce(bcolacc, bcol, "bcol")
                _refl_units(rh2s, racc2s, colacc2, t2, 2, UPC)
                _creduce(colacc2, ccol2, "ccol2")
                if not (no_accum or mm_only):
                    nc.sync.dma_start(out=racc1[:, :], in_=racc1s)
                    nc.sync.dma_start(out=racc2[:, :], in_=racc2s)
                    nc.sync.dma_start(out=bacc_d[:, :], in_=baccs)

            if loop is not None:
                with tc.For_i(0, loop):
                    _main_body()
            else:
                for _rep in range(repeat):
                    _main_body()

    nc.compile()
    return nc


def _get_nc(repeat=1, loop=None, **kw):
    key = ("nc", repeat, loop, tuple(sorted(kw.items())))
    if key not in _CACHE:
        _CACHE[key] = _build(repeat, loop=loop, **kw)
    return _CACHE[key]


def _host_prep(z1, z2):
    fp8 = ml_dtypes.float8_e4m3
    z1 = np.asarray(z1, dtype=np.float32)
    z2 = np.asarray(z2, dtype=np.float32)
    n1 = np.maximum(np.linalg.norm(z1, axis=1), 1e-12)
    n2 = np.maximum(np.linalg.norm(z2, axis=1), 1e-12)
    h1 = z1 / n1[:, None]
    h2 = z2 / n2[:, None]
    r1_8 = (h1 * RS).astype(fp8)
    r2_8 = (h2 * RS).astype(fp8)
    return r1_8, r2_8, h1, h2


def make_in_maps(z1, z2):
    r1_8, r2_8, _, _ = _host_prep(z1, z2)
    d1 = np.concatenate([r1_8.T, r1_8.T], axis=1)
    d2 = np.concatenate([r2_8.T, r2_8.T], axis=1)
    in_maps = []
    for r in range(NCORES):
        o = P * r
        in_maps.append({
            "rhw1": np.ascontiguousarray(d1[:, o:o + WN]),
            "rhw2": np.ascontiguousarray(d2[:, o:o + WN]),
        })
    return in_maps


def kernel(z1, z2):
    from concourse.bass_utils import run_bass_kernel_spmd

    r1_8, r2_8, h1, h2 = _host_prep(z1, z2)
    in_maps = make_in_maps(z1, z2)

    nc = _get_nc()
    res = run_bass_kernel_spmd(nc, in_maps, core_ids=list(range(NCORES)))

    S1 = np.zeros((NB, P), dtype=np.float64)
    S2 = np.zeros((NB, P), dtype=np.float64)
    Sb1 = np.zeros((NB, P), dtype=np.float64)
    C1 = np.zeros(N, dtype=np.float64)
    C2 = np.zeros(N, dtype=np.float64)
    Sb2 = np.zeros(N, dtype=np.float64)
    for r in range(NCORES):
        out = res.results[r]
        ra1 = out["racc1"].astype(np.float64).reshape(P, UPC, 3).sum(-1)
        ra2 = out["racc2"].astype(np.float64).reshape(P, UPC, 3).sum(-1)
        ba = out["bacc"].astype(np.float64).reshape(P, UPC, 4).sum(-1)
        for u in range(UPC):
            a = r + NCORES * u
            S1[a] += ra1[:, u]
            S2[a] += ra2[:, u]
            Sb1[a] += ba[:, u]
        C1 += np.roll(out["ccol1"].astype(np.float64), P * r)
        C2 += np.roll(out["ccol2"].astype(np.float64), P * r)
        Sb2 += np.roll(out["bcol"].astype(np.float64), P * r)

    S1 = S1.reshape(-1) + C1
    S2 = S2.reshape(-1) + C2
    Sb1 = Sb1.reshape(-1)

    # exact diagonal corrections from the same fp8 data the device used
    r1f = r1_8.astype(np.float64)
    r2f = r2_8.astype(np.float64)
    q1 = CEXP * (r1f * r1f).sum(1)
    q2 = CEXP * (r2f * r2f).sum(1)
    v5 = (h1.astype(np.float64) * h2.astype(np.float64)).sum(1) / TAU

    d1 = S1 + Sb1 - np.exp(q1)
    d2 = S2 + Sb2 - np.exp(q2)
    loss = 0.5 * (np.log(d1) + np.log(d2)) - v5
    return np.float32(loss.sum())
